# revision 1
# baseline (speedup 1.0000x reference)
"""CRF loss (forward-algorithm normalizer + tag-sequence score) on 8 trn2 cores.

Math
----
reference loss = sum_b (orig[y[b,0]] + sum_t trans[y[b,t],y[b,t+1]] - normalizer[b])
normalizer[b]  = sum_j alpha_{tau_b}[j, b],  tau_b = batch_sizes[b]-1
alpha_t[j, b]  = x_t[j, b] + logsumexp_k(alpha_{t-1}[k, b] + trans[j, k]),
alpha_0        = x_0 + orig.

Device recursion runs in the exp domain: with ea_t = exp(alpha_t - D_t[b])
(per-batch running offset D), the step becomes a plain matmul + one
elementwise multiply:

    S_t  = ETT_aug @ ea_{t-1}          # ETT[k, j] = exp(trans[j, k]); extra
                                       # ones-column gives row 64 = sigma =
                                       # sum_k ea_{t-1}[k, b]
    ea_t = exp(x_t) * S_t * r_t        # r_t = 1/sigma (applied every REN
                                       # steps, identity otherwise)
    D_t  = D_{t-1} - ln r_t            # only on renorm steps

All per-b scalars used for renormalization are *recorded* (recip rows), so
the final normalizer is exact regardless of which factor was applied:
    normalizer[b] = sum_j ln ea_tau[j, b] + C * D_tau[b].

The tag-score side is a single table gather: idx = y[b,t]*C + y[b,t+1] (plus
C*C+y[b,0] for the origination term) into concat(trans.ravel(), orig),
summed on device.

Sharding: data-parallel over batch, 64 rows per core; tiny parameters
replicated; per-core partial losses summed on the host.
"""

import sys

sys.path.insert(0, "/opt/trn_rl_repo")

import numpy as np

import concourse.bass as bass
import concourse.tile as tile
from concourse import bacc, mybir
from concourse.bass_utils import run_bass_kernel_spmd

# Problem constants (hardcoded per the task contract).
B, T, C = 512, 512, 64
M = 8            # cores
BL = B // M      # 64 batch rows per core
NG = 2           # independent pipelined groups per core
GW = BL // NG    # batch columns per group
REN = 4          # renormalize every REN steps
RQ = 32          # column blocks in recip history: events packed 4/quadrant
RSCALE = 2.0 ** -16  # extra renorm down-scale: keeps ea below the ACT Ln
                     # table's valid input range (2^64); exact power of two,
                     # so the recorded factor matches the applied one
CHUNK = 32       # timesteps of x per DMA chunk
TAB = C * C + C  # gather table size (4096 trans + 64 orig)
PAD_VAL = -1

f32 = mybir.dt.float32
bf16 = mybir.dt.bfloat16
AF = mybir.ActivationFunctionType
ALU = mybir.AluOpType

_CACHE = {}


def _renorm_steps():
    # Renorm at t in {REN, 2*REN, ...}; needs S_{t-2} so t >= 3; last t is 511.
    return [t for t in range(REN, T, REN)]


def build_program(bench_reps=1):
    """bench_reps > 1 wraps the recursion in a hardware loop; used only by
    the benchmark harness to amortize dispatch overhead. The product path
    (kernel()) always uses bench_reps=1."""
    key = ("nc", bench_reps)
    if key in _CACHE:
        return _CACHE[key]
    nc = bacc.Bacc("TRN2", target_bir_lowering=False, debug=False)

    xT = nc.declare_dram_parameter("xT", [C, T * BL], f32, isOutput=False)
    trT = nc.declare_dram_parameter("trT", [C, C], f32, isOutput=False)
    orig = nc.declare_dram_parameter("orig", [C, 1], f32, isOutput=False)
    tab = nc.declare_dram_parameter("tab", [128, TAB], f32, isOutput=False)
    pidx = nc.declare_dram_parameter("pidx", [128, 256], mybir.dt.uint16, isOutput=False)
    sidx = nc.declare_dram_parameter("sidx", [128, 4], mybir.dt.uint16, isOutput=False)
    parm = nc.declare_dram_parameter("parm", [128, BL], f32, isOutput=False)
    cutm = nc.declare_dram_parameter("cutm", [128, RQ * BL], f32, isOutput=False)
    res = nc.declare_dram_parameter("res", [1, 3], f32, isOutput=True)

    renorms = set(_renorm_steps())

    with tile.TileContext(nc) as tc:
        with (
            tc.tile_pool(name="const", bufs=1) as const,
            tc.tile_pool(name="hist", bufs=1) as histp,
            tc.tile_pool(name="x", bufs=3) as xpool,
            tc.tile_pool(name="w", bufs=2 * NG + 2) as wpool,
            tc.tile_pool(name="post", bufs=1) as post,
            tc.tile_pool(name="ps", bufs=2, space="PSUM") as psum,
            tc.tile_pool(name="psr", bufs=1, space="PSUM") as psumr,
        ):
            # ---- constants ----
            trT_s = const.tile([C, C], f32, tag="trT")
            nc.sync.dma_start(trT_s[:], trT[:])
            orig_s = const.tile([C, 1], f32, tag="orig")
            nc.sync.dma_start(orig_s[:], orig[:])
            # indirect_copy (gpsimd ISA) supports a single sync-wait, so all
            # of its inputs must be written by one engine: stage the DMA-landed
            # tiles through DVE copies.
            tab_r = const.tile([128, TAB], f32, tag="tab_r")
            nc.sync.dma_start(tab_r[:], tab[:])
            tab_s = const.tile([128, TAB], f32, tag="tab")
            nc.vector.tensor_copy(tab_s[:], tab_r[:])
            pidx_r = const.tile([128, 256], mybir.dt.uint16, tag="pidx_r")
            nc.sync.dma_start(pidx_r[:], pidx[:])
            pidx_s = const.tile([128, 256], mybir.dt.uint16, tag="pidx")
            nc.vector.tensor_copy(pidx_s[:], pidx_r[:])
            sidx_r = const.tile([128, 4], mybir.dt.uint16, tag="sidx_r")
            nc.sync.dma_start(sidx_r[:], sidx[:])
            sidx_s = const.tile([128, 4], mybir.dt.uint16, tag="sidx")
            nc.vector.tensor_copy(sidx_s[:], sidx_r[:])
            parm_s = const.tile([128, BL], f32, tag="parm")
            nc.sync.dma_start(parm_s[:], parm[:])
            cutm_s = const.tile([128, RQ * BL], f32, tag="cutm")
            nc.sync.dma_start(cutm_s[:], cutm[:])

            # ETT_aug[k, 0:C] = exp(trans[j=col, k=row]); ETT_aug[:, C] = 1.
            # Replicated in both partition halves: the recursion state for
            # step t lives in partition half t%2, and matmul operands must
            # share a base partition.
            ett = const.tile([128, C + 1], bf16, tag="ett")
            nc.scalar.activation(ett[0:C, 0:C], trT_s[:], AF.Exp)
            nc.scalar.activation(ett[C:128, 0:C], trT_s[:], AF.Exp)
            nc.vector.memset(ett[0:C, C : C + 1], 1.0)
            nc.vector.memset(ett[C:128, C : C + 1], 1.0)

            ones_row = const.tile([1, C], f32, tag="ones_row")
            nc.vector.memset(ones_row[:], RSCALE)
            ones_col128 = const.tile([128, 1], f32, tag="ones_col128")
            nc.vector.memset(ones_col128[:], 1.0)

            # recip history: event r lives at partition (r%4)*32, column
            # block r//4 (engine writes must start at a partition quadrant).
            # Preset to 1 so ln() of unused slots is 0.
            rhist = const.tile([128, RQ * BL], f32, tag="rhist")
            nc.vector.memset(rhist[:], 1.0)

            # bench-only iteration counter (res[0,2]); proves the For_i
            # actually looped when bench_reps > 1
            itc = const.tile([1, 1], f32, tag="itc")
            nc.vector.memset(itc[:], 0.0)

            # ea history: full recursion state. Step t lives at partition
            # half (t%2)*64, column block t//2 -- every slot gets written,
            # and consecutive steps alternate partition halves.
            hist = histp.tile([128, (T // 2) * BL], bf16, tag="hist")

            # ---- batch-score gather (independent of the recursion) ----
            gat = post.tile([128, 4096], f32, tag="gat")
            # ISA limit: <=1024 indices per indirect_copy
            for ip in range(4):
                nc.gpsimd.indirect_copy(
                    gat[:, 1024 * ip : 1024 * (ip + 1)],
                    tab_s[:],
                    pidx_s[:, 64 * ip : 64 * (ip + 1)],
                    True,
                )
            gsum = post.tile([128, 1], f32, tag="gsum")
            nc.vector.reduce_sum(gsum[:], gat[:], axis=mybir.AxisListType.X)
            btot = psumr.tile([1, 1], f32, tag="R0")
            nc.tensor.matmul(btot[:], ones_col128[:], gsum[:], start=True, stop=True)

            def hbase(t):
                return (t % 2) * 64

            def hcol(t):
                return (t // 2) * BL

            def emit_recursion():
                # ---- t = 0: ea_0 = exp(x_0 + orig) ----
                xc = xpool.tile([C, CHUNK * BL], f32, tag="xc")
                nc.sync.dma_start(xc[:], xT[:, 0 : CHUNK * BL])
                # one exp over the whole chunk; per-step W tiles are slices
                xe = xpool.tile([C, CHUNK * BL], f32, tag="xe")
                nc.scalar.activation(xe[:], xc[:], AF.Exp)
                # exp(x_0 + orig) = exp(x_0) * exp(orig): fold orig via a
                # per-partition scalar multiply, then DVE-copy into hist so
                # every hist write comes from DVE (indirect_copy wants a
                # single wait).
                eo = const.tile([C, 1], f32, tag="eo")
                nc.scalar.activation(eo[:], orig_s[:], AF.Exp)
                e0 = wpool.tile([C, BL], f32, tag="e0")
                nc.vector.tensor_scalar_mul(e0[:], xe[:, 0:BL], eo[:])
                nc.vector.tensor_copy(hist[0:C, 0:BL], e0[:])

                # ---- recursion ----
                S_prev = [[None, None] for _ in range(NG)]
                xecur = xe
                for t in range(1, T):
                    if t % CHUNK == 0:
                        xcur = xpool.tile([C, CHUNK * BL], f32, tag="xc")
                        nc.sync.dma_start(
                            xcur[:], xT[:, t * BL : (t + CHUNK) * BL]
                        )
                        xecur = xpool.tile([C, CHUNK * BL], f32, tag="xe")
                        nc.scalar.activation(xecur[:], xcur[:], AF.Exp)
                    xoff = (t % CHUNK) * BL

                    wt = None
                    if t in renorms:
                        # r = 1/sigma from S_{t-2} per group; record both
                        # halves with one copy, broadcast both with one
                        # matmul pair into a shared R tile, one fused W.
                        r_ev = t // REN - 1
                        rp = (r_ev % 4) * 32
                        rcol = (r_ev // 4) * BL
                        rrow = wpool.tile([1, BL], f32, tag="rr")
                        for g in range(NG):
                            Sold = S_prev[g][1]
                            nc.vector.reciprocal(
                                rrow[0:1, g * GW : (g + 1) * GW],
                                Sold[C : C + 1, :],
                            )
                        nc.vector.tensor_scalar_mul(
                            rhist[rp : rp + 1, rcol : rcol + BL], rrow[:], RSCALE
                        )
                        Rb = psumr.tile([C, BL], f32, tag="Rb")
                        nc.tensor.matmul(
                            Rb[:], ones_row[:], rrow[:], start=True, stop=True
                        )
                        wt = wpool.tile([C, BL], f32, tag="wt")
                        nc.vector.tensor_mul(
                            wt[:], xecur[:, xoff : xoff + BL], Rb[:]
                        )

                    for g in range(NG):
                        lo = g * GW
                        S = psum.tile([C + 1, GW], f32, tag=f"S{g}")
                        pb, cb = hbase(t - 1), hcol(t - 1) + lo
                        nc.tensor.matmul(
                            S[:],
                            ett[pb : pb + C, :],
                            hist[pb : pb + C, cb : cb + GW],
                            start=True,
                            stop=True,
                        )
                        if wt is not None:
                            win = wt[:, lo : lo + GW]
                        else:
                            win = xecur[:, xoff + lo : xoff + lo + GW]
                        # chain op: ea_t = S_t[0:C] * W
                        ob, oc = hbase(t), hcol(t) + lo
                        nc.vector.tensor_mul(
                            hist[ob : ob + C, oc : oc + GW],
                            S[0:C, :],
                            win,
                        )
                        S_prev[g][1] = S_prev[g][0]
                        S_prev[g][0] = S

            def emit_rep():
                nc.vector.tensor_scalar_add(itc[:], itc[:], 1.0)
                emit_recursion()

            if bench_reps == 1:
                emit_rep()
            else:
                with tc.For_i(0, bench_reps, 1):
                    emit_rep()

            # ---- final extraction ----
            snap = post.tile([128, BL], bf16, tag="snap")
            nc.gpsimd.indirect_copy(snap[:], hist[:], sidx_s[:], True)
            # both halves hold valid ea values (tau and its neighbor step);
            # parm selects the half that matches tau's parity.
            snapln = post.tile([128, BL], f32, tag="snapln")
            nc.scalar.activation(snapln[:], snap[:], AF.Ln)
            snapsel = post.tile([128, BL], f32, tag="snapsel")
            nc.vector.tensor_mul(snapsel[:], snapln[:], parm_s[:])

            lnr = post.tile([128, RQ * BL], f32, tag="lnr")
            nc.scalar.activation(lnr[:], rhist[:], AF.Ln)
            masked = post.tile([128, RQ * BL], f32, tag="masked")
            nc.vector.tensor_mul(masked[:], lnr[:], cutm_s[:])
            # sum the RQ column blocks: view [p, q*BL+b] as [p, b, q]
            sumq = post.tile([128, BL], f32, tag="sumq")
            mview = masked[:].rearrange("p (q b) -> p b q", q=RQ)
            nc.vector.reduce_sum(sumq[:], mview, axis=mybir.AxisListType.X)

            # normalizer[b] = sum_j snapsel[j, b] - C * sum_p sumq[p, b]
            nrowA = psumr.tile([1, BL], f32, tag="R1")
            nc.tensor.matmul(nrowA[:], ones_col128[:], snapsel[:], start=True, stop=True)
            nrowB = psumr.tile([1, BL], f32, tag="R0")
            nc.tensor.matmul(nrowB[:], ones_col128[:], sumq[:], start=True, stop=True)
            asum = post.tile([1, 1], f32, tag="asum")
            nc.vector.reduce_sum(asum[:], nrowA[:], axis=mybir.AxisListType.X)
            bsum = post.tile([1, 1], f32, tag="bsum")
            nc.vector.reduce_sum(bsum[:], nrowB[:], axis=mybir.AxisListType.X)

            out_s = post.tile([1, 3], f32, tag="out")
            nc.vector.tensor_copy(out_s[0:1, 2:3], itc[:])
            nc.vector.tensor_copy(out_s[0:1, 0:1], btot[:])
            nc.vector.scalar_tensor_tensor(
                out_s[0:1, 1:2], bsum[:], -float(C), asum[:],
                op0=ALU.mult, op1=ALU.add,
            )
            nc.sync.dma_start(res[:], out_s[:])

    nc.compile()
    _CACHE[key] = nc
    return nc


def host_inputs(pad_x, transition_scores, origination_scores, pad_y, batch_sizes):
    """Shard + lay out the full inputs into 8 per-core input maps."""
    pad_x = np.ascontiguousarray(np.asarray(pad_x, dtype=np.float32))
    trans = np.ascontiguousarray(np.asarray(transition_scores, dtype=np.float32))
    origv = np.ascontiguousarray(np.asarray(origination_scores, dtype=np.float32))
    pad_y = np.asarray(pad_y)
    batch_sizes = np.asarray(batch_sizes)

    # x transposed per core: xT[c][k, t*BL + b] = pad_x[c*BL + b, t, k]
    xr = pad_x.reshape(M, BL, T, C).transpose(0, 3, 2, 1)
    xT = np.ascontiguousarray(xr).reshape(M, C, T * BL)

    trT = np.ascontiguousarray(trans.T)
    orig = np.ascontiguousarray(origv.reshape(C, 1))

    tabv = np.concatenate([trans.reshape(-1), origv]).astype(np.float32)
    tab = np.ascontiguousarray(np.broadcast_to(tabv, (128, TAB)))

    y = np.where(pad_y == PAD_VAL, 0, pad_y).astype(np.int64)
    tau = batch_sizes.astype(np.int64) - 1

    # cut-mask event times: event r at t = REN*(r+1), stored at partition
    # (r%4)*32, column block r//4. t_r[q, p] for the 128-partition layout:
    # rows not in {0,32,64,96} never hold an event -> time inf (mask 0).
    t_r = np.full((RQ, 128), 10**9, dtype=np.int64)
    for r in range(len(_renorm_steps())):
        t_r[r // 4, (r % 4) * 32] = REN * (r + 1)

    in_maps = []
    for c in range(M):
        yc = y[c * BL : (c + 1) * BL]
        pair = (yc[:, :-1] * C + yc[:, 1:]).reshape(-1)
        oidx = C * C + yc[:, 0]
        allidx = np.concatenate([pair, oidx])  # 32768 entries
        pidx = np.zeros((128, 256), np.uint16)
        for gc in range(8):
            blk = allidx[4096 * gc : 4096 * (gc + 1)].reshape(256, 16)
            pidx[16 * gc : 16 * (gc + 1), :] = blk.T.astype(np.uint16)

        tauc = tau[c * BL : (c + 1) * BL]
        si = ((tauc // 2) * BL + np.arange(BL)).astype(np.uint16)
        sblk = si.reshape(4, 16).T  # [16, 4]
        sidx = np.ascontiguousarray(np.tile(sblk, (8, 1)))
        # parity mask: tau even -> rows 0..63, tau odd -> rows 64..127
        par = np.zeros((128, BL), np.float32)
        par[:64, :] = (tauc % 2 == 0).astype(np.float32)[None, :]
        par[64:, :] = (tauc % 2 == 1).astype(np.float32)[None, :]

        cut = (t_r[:, :, None] <= tauc[None, None, :]).astype(np.float32)
        cutm = np.ascontiguousarray(cut.transpose(1, 0, 2).reshape(128, RQ * BL))

        in_maps.append(
            {
                "xT": np.ascontiguousarray(xT[c]),
                "trT": trT,
                "orig": orig,
                "tab": tab,
                "pidx": pidx,
                "sidx": sidx,
                "parm": par,
                "cutm": cutm,
            }
        )
    return in_maps


def combine(results):
    total = 0.0
    for r in results:
        v = np.asarray(r["res"], dtype=np.float64).reshape(-1)
        total += v[0] / 16.0 - v[1]
    return np.asarray(total, dtype=np.float32)


def kernel(pad_x, transition_scores, origination_scores, pad_y, batch_sizes):
    nc = build_program()
    in_maps = host_inputs(
        pad_x, transition_scores, origination_scores, pad_y, batch_sizes
    )
    out = run_bass_kernel_spmd(nc, in_maps, core_ids=list(range(M)))
    return combine(out.results)



# revision 17
# speedup vs baseline: 1.7708x; 1.7708x over previous
"""CRF loss (forward-algorithm normalizer + tag-sequence score) on 8 trn2 cores.

Math
----
reference loss = sum_b (orig[y[b,0]] + sum_t trans[y[b,t],y[b,t+1]] - normalizer[b])
normalizer[b]  = sum_j alpha_{tau_b}[j, b],  tau_b = batch_sizes[b]-1
alpha_t[j, b]  = x_t[j, b] + logsumexp_k(alpha_{t-1}[k, b] + trans[j, k]),
alpha_0        = x_0 + orig.

Device recursion runs in the exp domain: with ea_t = exp(alpha_t - D_t[b]),
the step is one matmul + one elementwise multiply:

    S_t  = ETT @ ea_{t-1}              # ETT[k, j] = exp(trans[j, k])
    ea_t = exp(x_t) * S_t * r_t        # r_t = RSCALE/sigma every REN steps
    D_t  = D_{t-1} - ln r_t            # recorded exactly via rhist (bf16)

Per-core layout stacks two 32-column batch chains on the 128 partitions
(chain a=0 on partitions 0:64, a=1 on 64:128) with a block-diagonal ETT,
so each timestep is a single [128x128]x[128,32] matmul and a single
[128,32] DVE multiply -- the serial chain is latency-bound, so fewer,
uniform instructions per step win.

The tag-score side is computed from host-built integer histograms
(count[next, cur] of transition pairs, plus first-tag counts): the device
does sum(count * ptab) where ptab = [trans^T | orig]; parameters are only
ever touched on device.

Sharding: data-parallel over batch, 64 rows per core; per-core partial
sums combined on the host (pure index constants only).
"""

import sys

sys.path.insert(0, "/opt/trn_rl_repo")

import numpy as np
import ml_dtypes

import concourse.bass as bass
import concourse.tile as tile
from concourse import bacc, mybir
from concourse.bass_utils import run_bass_kernel_spmd

# Problem constants (hardcoded per the task contract).
B, T, C = 512, 512, 64
M = 8            # cores
BL = B // M      # 64 batch rows per core
GW = 32          # columns per chain (2 chains stacked on partitions)
REN = 8          # renormalize every REN steps
NEVT = T // REN - 1          # 63 renorm events (t = 8, 16, ..., 504)
RQ = NEVT        # column blocks in recip history (one event per block, all
                 # slots at partition base 0 -- PE operands off base 0 fault)
RSCALE = 2.0 ** -32          # renorm down-scale, exact in bf16
LN_RSCALE = float(-32.0 * np.log(2.0))
CHUNK = 32       # timesteps of x per DMA chunk
SUB = 8          # timesteps per exp sub-activation
PAD_VAL = -1

f32 = mybir.dt.float32
bf16 = mybir.dt.bfloat16
u16 = mybir.dt.uint16
AF = mybir.ActivationFunctionType
ALU = mybir.AluOpType

_CACHE = {}


def build_program(bench_reps=1):
    key = ("nc", bench_reps)
    if key in _CACHE:
        return _CACHE[key]
    nc = bacc.Bacc("TRN2", target_bir_lowering=False, debug=False)

    xT = nc.declare_dram_parameter("xT", [128, T * GW], f32, isOutput=False)
    trT = nc.declare_dram_parameter("trT", [C, C], f32, isOutput=False)
    orig128 = nc.declare_dram_parameter("orig128", [128, 1], f32, isOutput=False)
    ptab = nc.declare_dram_parameter("ptab", [C, C + 1], f32, isOutput=False)
    cnt = nc.declare_dram_parameter("cnt", [C, C + 1], f32, isOutput=False)
    sidx = nc.declare_dram_parameter("sidx", [128, 4], u16, isOutput=False)
    parm = nc.declare_dram_parameter("parm", [128, BL], f32, isOutput=False)
    cutm = nc.declare_dram_parameter("cutm", [128, RQ * GW], f32, isOutput=False)
    b2d = nc.declare_dram_parameter("b2d", [128, 128], bf16, isOutput=False)
    ones2d = nc.declare_dram_parameter("ones2d", [128, 2], bf16, isOutput=False)
    res = nc.declare_dram_parameter("res", [1, 4], f32, isOutput=True)

    NCH = T // CHUNK

    with tile.TileContext(nc) as tc:
        with (
            tc.tile_pool(name="const", bufs=1) as const,
            tc.tile_pool(name="hist", bufs=1) as histp,
            tc.tile_pool(name="xc", bufs=3) as xcpool,
            tc.tile_pool(name="xe", bufs=3) as xepool,
            tc.tile_pool(name="w", bufs=2) as wpool,
            tc.tile_pool(name="post", bufs=1) as post,
            tc.tile_pool(name="psS", bufs=2, space="PSUM") as psS,
            tc.tile_pool(name="psSig", bufs=1, space="PSUM") as psSig,
            tc.tile_pool(name="psRb", bufs=1, space="PSUM") as psRb,
            tc.tile_pool(name="psFin", bufs=1, space="PSUM") as psFin,
        ):
            # ---- constants ----
            trT_s = const.tile([C, C], f32, tag="trT")
            nc.sync.dma_start(trT_s[:], trT[:])
            orig_s = const.tile([128, 1], f32, tag="orig")
            nc.sync.dma_start(orig_s[:], orig128[:])
            ptab_s = const.tile([C, C + 1], f32, tag="ptab")
            nc.sync.dma_start(ptab_s[:], ptab[:])
            cnt_s = const.tile([C, C + 1], f32, tag="cnt")
            nc.sync.dma_start(cnt_s[:], cnt[:])
            sidx_r = const.tile([128, 4], u16, tag="sidx_r")
            nc.sync.dma_start(sidx_r[:], sidx[:])
            sidx_s = const.tile([128, 4], u16, tag="sidx")
            nc.vector.tensor_copy(sidx_s[:], sidx_r[:])
            parm_s = const.tile([128, BL], f32, tag="parm")
            nc.sync.dma_start(parm_s[:], parm[:])
            cutm_s = const.tile([128, RQ * GW], f32, tag="cutm")
            nc.sync.dma_start(cutm_s[:], cutm[:])
            b2_s = const.tile([128, 128], bf16, tag="b2")
            nc.sync.dma_start(b2_s[:], b2d[:])
            ones2_s = const.tile([128, 2], bf16, tag="ones2")
            nc.sync.dma_start(ones2_s[:], ones2d[:])

            # Block-diagonal exp(trans)^T: chain a on partitions a*64..a*64+63.
            ett = const.tile([128, 128], bf16, tag="ett")
            nc.vector.memset(ett[:], 0.0)
            nc.scalar.activation(ett[0:C, 0:C], trT_s[:], AF.Exp)
            nc.scalar.activation(ett[C:128, C:128], trT_s[:], AF.Exp)

            eo = const.tile([128, 1], f32, tag="eo")
            nc.scalar.activation(eo[:], orig_s[:], AF.Exp)

            ones_col = const.tile([128, 1], f32, tag="ones_col")
            nc.vector.memset(ones_col[:], 1.0)

            # recip history: event r at partitions (r%4)*32 + {0,1}, column
            # block r//4.  Preset 1 so Ln of unused slots is 0.
            rhist = const.tile([128, RQ * GW], bf16, tag="rhist")
            nc.vector.memset(rhist[:], 1.0)

            itc = const.tile([1, 1], f32, tag="itc")
            nc.vector.memset(itc[:], 0.0)

            # ea history: step t at columns [t*GW, (t+1)*GW).
            hist = histp.tile([128, T * GW], bf16, tag="hist")

            # ---- tag-score from host histograms (independent of recursion) ----
            gmul = post.tile([C, C + 1], f32, tag="gmul")
            gacc = post.tile([C, 1], f32, tag="gacc")
            nc.vector.scalar_tensor_tensor(
                gmul[:], cnt_s[:], 1.0, ptab_s[:],
                op0=ALU.mult, op1=ALU.mult, accum_out=gacc[:],
            )
            btot = psFin.tile([1, 1], f32, tag="Rg")
            nc.tensor.matmul(btot[:], ones_col[0:C, :], gacc[:], start=True, stop=True)

            def emit_recursion():
                chunk_tiles = {}

                def emit_chunk_dma(ci):
                    xc = xcpool.tile([128, CHUNK * GW], f32, tag="xc")
                    nc.sync.dma_start(xc[:], xT[:, ci * CHUNK * GW : (ci + 1) * CHUNK * GW])
                    xe = xepool.tile([128, CHUNK * GW], bf16, tag="xe")
                    chunk_tiles[ci] = (xc, xe)

                def emit_subexp(blk):
                    # blk covers steps [blk*SUB, (blk+1)*SUB)
                    if blk * SUB >= T:
                        return
                    ci, sub = divmod(blk, CHUNK // SUB)
                    xc, xe = chunk_tiles[ci]
                    lo, hi = sub * SUB * GW, (sub + 1) * SUB * GW
                    nc.scalar.activation(xe[:, lo:hi], xc[:, lo:hi], AF.Exp)

                emit_chunk_dma(0)
                emit_chunk_dma(1)
                for b in range(3):
                    emit_subexp(b)

                # ---- t = 0: ea_0 = exp(x_0) * exp(orig) ----
                xe0 = chunk_tiles[0][1]
                nc.vector.tensor_scalar_mul(hist[:, 0:GW], xe0[:, 0:GW], eo[:])

                wt = None
                for t in range(1, T):
                    ci, off = divmod(t, CHUNK)
                    if off == 0 and ci + 1 < NCH:
                        emit_chunk_dma(ci + 1)
                    if t % SUB == 0:
                        emit_subexp(t // SUB + 2)

                    xecur = chunk_tiles[ci][1]
                    xoff = off * GW

                    if t % REN == 0 and t <= REN * NEVT:
                        win = wt[:]
                    else:
                        win = xecur[:, xoff : xoff + GW]

                    S = psS.tile([128, GW], f32, tag="S")
                    nc.tensor.matmul(
                        S[:], ett[:], hist[:, (t - 1) * GW : t * GW],
                        start=True, stop=True,
                    )
                    nc.vector.tensor_mul(hist[:, t * GW : (t + 1) * GW], S[:], win)

                    # renorm prep for event at te = t+2 / te = t+1 (off-chain)
                    if t % REN == REN - 2 and t + 2 <= REN * NEVT:
                        r = (t + 2) // REN - 1
                        rp, rcol = 0, r * GW
                        sig = psSig.tile([2, GW], f32, tag="sig")
                        nc.tensor.matmul(
                            sig[:], ones2_s[:], hist[:, t * GW : (t + 1) * GW],
                            start=True, stop=True,
                        )
                        rr = wpool.tile([2, GW], f32, tag="rr")
                        nc.vector.reciprocal(rr[:], sig[:])
                        nc.vector.tensor_copy(
                            rhist[rp : rp + 2, rcol : rcol + GW], rr[:]
                        )
                    if t % REN == REN - 1 and t + 1 <= REN * NEVT:
                        r = (t + 1) // REN - 1
                        rp, rcol = 0, r * GW
                        Rb = psRb.tile([128, GW], f32, tag="Rb")
                        nc.tensor.matmul(
                            Rb[:],
                            b2_s[rp : rp + 2, :],
                            rhist[rp : rp + 2, rcol : rcol + GW],
                            start=True, stop=True,
                        )
                        nci, noff = divmod(t + 1, CHUNK)
                        wt = wpool.tile([128, GW], bf16, tag="wt")
                        nc.vector.tensor_mul(
                            wt[:],
                            chunk_tiles[nci][1][:, noff * GW : (noff + 1) * GW],
                            Rb[:],
                        )

            def emit_rep():
                nc.vector.tensor_scalar_add(itc[:], itc[:], 1.0)
                emit_recursion()

            if bench_reps == 1:
                emit_rep()
            else:
                with tc.For_i(0, bench_reps, 1):
                    emit_rep()

            # ---- final extraction ----
            # snap[:, b] = hist[:, tau_b*GW + (b%32)]; valid half selected by parm.
            snap = post.tile([128, BL], bf16, tag="snap")
            nc.gpsimd.indirect_copy(snap[:], hist[:], sidx_s[:], True)
            snapln = post.tile([128, BL], f32, tag="snapln")
            nc.scalar.activation(snapln[:], snap[:], AF.Ln)
            snapsel = post.tile([128, BL], f32, tag="snapsel")
            sacc = post.tile([128, 1], f32, tag="sacc")
            nc.vector.scalar_tensor_tensor(
                snapsel[:], snapln[:], 1.0, parm_s[:],
                op0=ALU.mult, op1=ALU.mult, accum_out=sacc[:],
            )

            lnr = post.tile([128, RQ * GW], f32, tag="lnr")
            nc.scalar.activation(lnr[:], rhist[:], AF.Ln)
            masked = post.tile([128, RQ * GW], f32, tag="masked")
            racc = post.tile([128, 1], f32, tag="racc")
            nc.vector.scalar_tensor_tensor(
                masked[:], lnr[:], 1.0, cutm_s[:],
                op0=ALU.mult, op1=ALU.mult, accum_out=racc[:],
            )

            nA = psFin.tile([1, 1], f32, tag="RA")
            nc.tensor.matmul(nA[:], ones_col[:], sacc[:], start=True, stop=True)
            nB = psFin.tile([1, 1], f32, tag="RB")
            nc.tensor.matmul(nB[:], ones_col[:], racc[:], start=True, stop=True)

            out_s = post.tile([1, 4], f32, tag="out")
            nc.vector.tensor_copy(out_s[0:1, 0:1], btot[:])
            nc.vector.tensor_copy(out_s[0:1, 1:2], nA[:])
            nc.vector.tensor_copy(out_s[0:1, 2:3], nB[:])
            nc.vector.tensor_copy(out_s[0:1, 3:4], itc[:])
            nc.sync.dma_start(res[:], out_s[:])

    nc.compile()
    _CACHE[key] = nc
    return nc


def host_inputs(pad_x, transition_scores, origination_scores, pad_y, batch_sizes):
    """Shard + lay out the full inputs into 8 per-core input maps.

    Host work is limited to data movement and integer index preprocessing;
    every floating-point op on learned parameters / activations runs on
    device.  Returns (in_maps, nev_consts)."""
    pad_x = np.ascontiguousarray(np.asarray(pad_x, dtype=np.float32))
    trans = np.ascontiguousarray(np.asarray(transition_scores, dtype=np.float32))
    origv = np.ascontiguousarray(np.asarray(origination_scores, dtype=np.float32))
    pad_y = np.asarray(pad_y)
    batch_sizes = np.asarray(batch_sizes)

    # x: xT[c][a*64 + k, t*32 + cc] = pad_x[c*64 + a*32 + cc, t, k]
    xr = pad_x.reshape(M, 2, GW, T, C).transpose(0, 1, 4, 3, 2)
    xT = np.ascontiguousarray(xr).reshape(M, 128, T * GW)

    trT = np.ascontiguousarray(trans.T)
    orig128 = np.ascontiguousarray(
        np.concatenate([origv, origv]).reshape(128, 1)
    )
    ptab = np.ascontiguousarray(
        np.concatenate([trans.T, origv.reshape(C, 1)], axis=1)
    )

    y = np.where(pad_y == PAD_VAL, 0, pad_y).astype(np.int64)
    tau = batch_sizes.astype(np.int64) - 1

    # constant bf16 matmul helpers
    b2 = np.zeros((128, 128), np.float32)
    b2[0, 0:64] = RSCALE
    b2[1, 64:128] = RSCALE
    b2 = b2.astype(ml_dtypes.bfloat16)
    ones2 = np.zeros((128, 2), np.float32)
    ones2[0:64, 0] = 1.0
    ones2[64:128, 1] = 1.0
    ones2 = ones2.astype(ml_dtypes.bfloat16)

    parm = np.zeros((128, BL), np.float32)
    for a in range(2):
        parm[a * 64 : (a + 1) * 64, a * GW : (a + 1) * GW] = 1.0

    in_maps = []
    nevs = []
    for c in range(M):
        yc = y[c * BL : (c + 1) * BL]
        # count[next, cur] histogram + first-tag histogram (integer only)
        pair = (yc[:, 1:] * C + yc[:, :-1]).reshape(-1)
        cntm = np.bincount(pair, minlength=C * C).astype(np.float32).reshape(C, C)
        ho = np.bincount(yc[:, 0], minlength=C).astype(np.float32).reshape(C, 1)
        cnt = np.ascontiguousarray(np.concatenate([cntm, ho], axis=1))

        tauc = tau[c * BL : (c + 1) * BL]
        idx64 = (tauc * GW + (np.arange(BL) % GW)).astype(np.uint16)
        blk = idx64.reshape(4, 16).T  # wrapped per 16 partitions
        sidx = np.ascontiguousarray(np.tile(blk, (8, 1)))

        # cutm[a, r*GW+cc] = 1 iff event r has 8*(r+1) <= tau of batch col
        # b = a*GW + cc
        cutm = np.zeros((128, RQ * GW), np.float32)
        for r in range(NEVT):
            t_r = REN * (r + 1)
            rp, rcol = 0, r * GW
            for a in range(2):
                bvals = tauc[a * GW : (a + 1) * GW]
                cutm[rp + a, rcol : rcol + GW] = (t_r <= bvals).astype(np.float32)

        nevs.append(float(np.minimum(tau[c * BL : (c + 1) * BL] // REN, NEVT).sum()))

        in_maps.append(
            {
                "xT": np.ascontiguousarray(xT[c]),
                "trT": trT,
                "orig128": orig128,
                "ptab": ptab,
                "cnt": cnt,
                "sidx": sidx,
                "parm": parm,
                "cutm": cutm,
                "b2d": b2,
                "ones2d": ones2,
            }
        )
    return in_maps, nevs


def combine(results, nevs):
    total = 0.0
    for r, nev in zip(results, nevs):
        v = np.asarray(r["res"], dtype=np.float64).reshape(-1)
        # loss_core = score - sum_b normalizer_b
        #           = v0 - (v1 - C*(v2 + ln(RSCALE)*nev))
        total += v[0] - v[1] + C * (v[2] + LN_RSCALE * nev)
    return np.asarray(total, dtype=np.float32)


def kernel(pad_x, transition_scores, origination_scores, pad_y, batch_sizes):
    nc = build_program()
    in_maps, nevs = host_inputs(
        pad_x, transition_scores, origination_scores, pad_y, batch_sizes
    )
    out = run_bass_kernel_spmd(nc, in_maps, core_ids=list(range(M)))
    return combine(out.results, nevs)
